# revision 2
# baseline (speedup 1.0000x reference)
"""NonLocal2D block (SAGAN-style non-local attention) on 8 Trainium2 cores.

Data-parallel over batch: core b computes batch element b entirely on-chip.

Math (per batch, N = 64*64 = 4096):
  f = Wf@x+bf [16,N], g = Wg@x+bg [16,N], h = Wh@x+bh [128,N]
  S = f^T g [N,N]; A = softmax_rows(S); att = h @ A; out = x + gamma*att

Decomposition (per core), using att[c,m] = sum_n hT'[n,c] * E[n,m] with
E = exp(S) and hT'[n,c] = (h[c,n]) * gamma/D[n], D[n] = sum_m E[n,m]:

  32 row-strips of 128 n's. Per strip:
    S_strip = f_strip^T @ g          PE, K=16 bf16 matmuls -> PSUM
    E_strip = exp(S_strip)           ACT (the roofline: 16.7M exps ~ 110us),
                                     PSUM->SBUF bf16; last 2 chunks also
                                     emit accum_out partial row-sums
    D rowsum                         DVE reduce (first 2048 cols) + ACT accums
    hT = x_strip^T @ WhT + 1 (x) bh  PE (K=128 + K=1 rank-1 bias)
    hT' = hT * (gamma/D)             DVE, PSUM->SBUF bf16
  Attended accumulates over 8-strip groups in PSUM (K-chained matmuls),
  folded to an SBUF accumulator by DVE, software-pipelined one group of
  windows behind production so ACT never starves. Tail: residual + store.
"""

import numpy as np
import ml_dtypes

import concourse.bass as bass
import concourse.bacc as bacc
import concourse.tile as tile
import concourse.mybir as mybir
from concourse.bass_utils import run_bass_kernel_spmd

B, C, W, H = 8, 128, 64, 64
N = W * H          # 4096
CP = 16            # f/g channels
P = 128
NSTRIP = N // P    # 32
WSTRIPS = 4        # strips per production window
NWIN = NSTRIP // WSTRIPS    # 8 windows
GROUP = 8          # strips per attended K-chain (= 2 windows)
MBLK = 512
NMB = N // MBLK    # 8
CHUNK = 1024       # exp call width (2 PSUM banks)
NCHUNK = N // CHUNK             # 4
DVE_CHUNKS = 2     # rowsum: chunks 0..1 on DVE, rest via ACT accum_out

F32 = mybir.dt.float32
BF16 = mybir.dt.bfloat16
EXP = mybir.ActivationFunctionType.Exp
AX = mybir.AxisListType.X
MUL = mybir.AluOpType.mult

_NC = None


def _build():
    nc = bacc.Bacc(None, target_bir_lowering=False)
    x32 = nc.dram_tensor("x32", [P, N], F32, kind="ExternalInput")
    xbf = nc.dram_tensor("xbf", [P, N], BF16, kind="ExternalInput")
    wft4 = nc.dram_tensor("wft4", [P, P], BF16, kind="ExternalInput")
    wgt4 = nc.dram_tensor("wgt4", [P, P], BF16, kind="ExternalInput")
    wht = nc.dram_tensor("wht", [P, P], BF16, kind="ExternalInput")
    bf4 = nc.dram_tensor("bf4", [P, 1], F32, kind="ExternalInput")
    bg4 = nc.dram_tensor("bg4", [P, 1], F32, kind="ExternalInput")
    bhr = nc.dram_tensor("bhr", [1, P], BF16, kind="ExternalInput")
    gam = nc.dram_tensor("gam", [1, 1], F32, kind="ExternalInput")
    out = nc.dram_tensor("out", [P, N], F32, kind="ExternalOutput")

    with tile.TileContext(nc) as tc:
        with (
            tc.tile_pool(name="consts", bufs=1) as consts,
            tc.tile_pool(name="epool", bufs=2 * GROUP) as epool,
            tc.tile_pool(name="hpool", bufs=2 * GROUP + 2) as hpool,
            tc.tile_pool(name="small", bufs=6) as small,
            tc.tile_pool(name="psS", bufs=2, space="PSUM") as psS,
            tc.tile_pool(name="psA", bufs=3, space="PSUM") as psA,
            tc.tile_pool(name="psH", bufs=1, space="PSUM") as psH,
        ):
            # ---- constants / inputs ----
            xbf_s = consts.tile([P, N], BF16)
            for j in range(NMB):
                nc.sync.dma_start(xbf_s[:, j * MBLK:(j + 1) * MBLK],
                                  xbf[:, j * MBLK:(j + 1) * MBLK])
            x32_s = consts.tile([P, N], F32)
            for j in range(NMB):
                nc.sync.dma_start(x32_s[:, j * MBLK:(j + 1) * MBLK],
                                  x32[:, j * MBLK:(j + 1) * MBLK])
            wft4_s = consts.tile([P, P], BF16)
            nc.sync.dma_start(wft4_s[:], wft4[:])
            wgt4_s = consts.tile([P, P], BF16)
            nc.sync.dma_start(wgt4_s[:], wgt4[:])
            wht_s = consts.tile([P, P], BF16)
            nc.sync.dma_start(wht_s[:], wht[:])
            bf4_s = consts.tile([P, 1], F32)
            nc.sync.dma_start(bf4_s[:], bf4[:])
            bg4_s = consts.tile([P, 1], F32)
            nc.sync.dma_start(bg4_s[:], bg4[:])
            bhr_s = consts.tile([1, P], BF16)
            nc.sync.dma_start(bhr_s[:], bhr[:])
            gam_s = consts.tile([P, 1], F32)
            nc.sync.dma_start(gam_s[:], gam[:].to_broadcast([P, 1]))
            ones_s = consts.tile([1, P], BF16)
            nc.vector.memset(ones_s[:], 1.0)

            f4 = consts.tile([P, N], BF16)
            g4 = consts.tile([P, N], BF16)
            att = consts.tile([P, N], F32)

            # ---- f/g 1x1 convs (K=128 matmuls); bias added on PSUM->SBUF copy
            for j in range(NMB):
                blk = slice(j * MBLK, (j + 1) * MBLK)
                psf = psA.tile([P, MBLK], F32, tag="att")
                nc.tensor.matmul(psf[:], wft4_s[:], xbf_s[:, blk],
                                 start=True, stop=True)
                nc.vector.tensor_scalar_add(out=f4[:, blk], in0=psf[:],
                                            scalar1=bf4_s[:])
                psg = psA.tile([P, MBLK], F32, tag="att")
                nc.tensor.matmul(psg[:], wgt4_s[:], xbf_s[:, blk],
                                 start=True, stop=True)
                nc.vector.tensor_scalar_add(out=g4[:, blk], in0=psg[:],
                                            scalar1=bg4_s[:])

            def att_block(j, group, first):
                """att[:, blk j] (+)= sum_k hT'_k^T @ E_k[:, blk j]."""
                blk = slice(j * MBLK, (j + 1) * MBLK)
                pa = psA.tile([P, MBLK], F32, tag="att")
                for k, (hk, ek) in enumerate(group):
                    nc.tensor.matmul(pa[:], hk[:], ek[:, blk],
                                     start=(k == 0), stop=(k == len(group) - 1))
                if first:
                    nc.vector.tensor_copy(out=att[:, blk], in_=pa[:])
                else:
                    nc.vector.tensor_add(out=att[:, blk], in0=att[:, blk],
                                         in1=pa[:])

            groups = [[] for _ in range(NSTRIP // GROUP)]
            for w in range(NWIN):
                psh = psH.tile([P, 4 * P], F32)  # one bank, 4 strips' hT
                for i in range(WSTRIPS):
                    s = w * WSTRIPS + i
                    sl = slice(s * P, (s + 1) * P)
                    # hT = x_strip^T @ WhT + ones (x) bh  -> [n, c] in PSUM
                    ph = psh[:, i * P:(i + 1) * P]
                    nc.tensor.matmul(ph, xbf_s[:, sl], wht_s[:],
                                     start=True, stop=False)
                    nc.tensor.matmul(ph, ones_s[:], bhr_s[:],
                                     start=False, stop=True)
                    # S strip (K=16) -> exp -> E strip (+ accum partial sums)
                    e = epool.tile([P, N], BF16, tag="E")
                    accs = small.tile([P, NCHUNK - DVE_CHUNKS], F32, tag="accs")
                    for cix in range(NCHUNK):
                        sps = psS.tile([P, CHUNK], F32)
                        for half in range(2):
                            mlo = cix * CHUNK + half * MBLK
                            nc.tensor.matmul(
                                sps[:, half * MBLK:(half + 1) * MBLK],
                                f4[0:CP, sl],
                                g4[0:CP, mlo:mlo + MBLK],
                                start=True, stop=True)
                        eout = e[:, cix * CHUNK:(cix + 1) * CHUNK]
                        if cix < DVE_CHUNKS:
                            nc.scalar.activation(out=eout, in_=sps[:], func=EXP)
                        else:
                            nc.scalar.activation(
                                out=eout, in_=sps[:], func=EXP,
                                accum_out=accs[:, cix - DVE_CHUNKS:
                                               cix - DVE_CHUNKS + 1])
                    # D rowsum -> rd = gamma / D
                    dd = small.tile([P, 1], F32, tag="dd")
                    nc.vector.reduce_sum(out=dd[:],
                                         in_=e[:, 0:DVE_CHUNKS * CHUNK], axis=AX)
                    d = small.tile([P, 1], F32, tag="d")
                    nc.vector.tensor_add(out=d[:], in0=dd[:],
                                         in1=accs[:, 0:1])
                    for a in range(1, NCHUNK - DVE_CHUNKS):
                        nc.vector.tensor_add(out=d[:], in0=d[:],
                                             in1=accs[:, a:a + 1])
                    rd = small.tile([P, 1], F32, tag="rd")
                    nc.vector.reciprocal(out=rd[:], in_=d[:])
                    hts = hpool.tile([P, P], BF16, tag="hts")
                    nc.vector.tensor_scalar(out=hts[:], in0=ph,
                                            scalar1=rd[:], scalar2=gam_s[:],
                                            op0=MUL, op1=MUL)
                    groups[s // GROUP].append((hts, e))
                    # attended for the group finished two windows ago (keeps
                    # PE busy while ACT chews the current window's exps)
                    if w >= 2:
                        k = w // 2 - 1
                        att_block(4 * (w % 2) + i, groups[k], first=(k == 0))

            # tail: attended for the last group, then residual + store
            for j in range(NMB):
                att_block(j, groups[-1], first=False)
                blk = slice(j * MBLK, (j + 1) * MBLK)
                nc.vector.tensor_add(out=att[:, blk], in0=att[:, blk],
                                     in1=x32_s[:, blk])
                nc.sync.dma_start(out[:, blk], att[:, blk])

    nc.compile()
    return nc


def _get_nc():
    global _NC
    if _NC is None:
        _NC = _build()
    return _NC


def _prep_weights(Wf, bf, Wg, bg, Wh, bh, gamma):
    bf16 = ml_dtypes.bfloat16
    wft4 = np.zeros((P, P), np.float32)
    wgt4 = np.zeros((P, P), np.float32)
    bf4 = np.zeros((P, 1), np.float32)
    bg4 = np.zeros((P, 1), np.float32)
    for i in range(4):
        wft4[:, 32 * i:32 * i + CP] = Wf.T
        wgt4[:, 32 * i:32 * i + CP] = Wg.T
        bf4[32 * i:32 * i + CP, 0] = bf
        bg4[32 * i:32 * i + CP, 0] = bg
    return {
        "wft4": wft4.astype(bf16),
        "wgt4": wgt4.astype(bf16),
        "wht": np.ascontiguousarray(Wh.T).astype(bf16),
        "bf4": bf4,
        "bg4": bg4,
        "bhr": np.ascontiguousarray(bh.reshape(1, P)).astype(bf16),
        "gam": np.asarray(gamma, np.float32).reshape(1, 1),
    }


def make_in_maps(x, Wf, bf, Wg, bg, Wh, bh, gamma):
    bf16 = ml_dtypes.bfloat16
    wmap = _prep_weights(np.asarray(Wf), np.asarray(bf), np.asarray(Wg),
                         np.asarray(bg), np.asarray(Wh), np.asarray(bh),
                         np.asarray(gamma))
    xf = np.ascontiguousarray(np.asarray(x, np.float32).reshape(B, C, N))
    in_maps = []
    for b in range(B):
        m = dict(wmap)
        m["x32"] = xf[b]
        m["xbf"] = xf[b].astype(bf16)
        in_maps.append(m)
    return in_maps


def kernel(x, Wf, bf, Wg, bg, Wh, bh, gamma):
    nc = _get_nc()
    in_maps = make_in_maps(x, Wf, bf, Wg, bg, Wh, bh, gamma)
    res = run_bass_kernel_spmd(nc, in_maps, core_ids=list(range(B)))
    out = np.stack([res.results[b]["out"] for b in range(B)], axis=0)
    return out.reshape(B, C, W, H).astype(np.float32)
